# revision 46
# baseline (speedup 1.0000x reference)
"""MemNN (embedding_lookup) Trainium2 Bass kernel.

Strategy (8 NeuronCores, one NEFF, SPMD):
  - Data-parallel hops: batch dim sharded 8 ways (8 batches/core).
  - Host packs the 4 embedding tables interleaved per vocab row
    ([A0|A1|A2|A3][v], bf16) and, per core, compacts it to the core's
    unique vocab rows so indices fit dma_gather's int16 (~22.6K < 32767).
  - dma_gather streams all (story + query) embedding rows across the
    4 SWDGE queues (round-robin) so descriptor generation runs on all
    4 Q7 core-pairs concurrently; PE matmuls with fixed block weights
    reduce each 128-row tile into per-sentence partial sums (PSUM).
    Position encoding is rank-2 separable:
        pe[j,d] = a(j) + b(j) * k'(d),  a=1-j/J, b=2j/J-1, k'=d/D
    so m = S_a + k' * S_b needs only two weighted sums per sentence.
    Temporal encodings are folded into the per-chunk combines.
  - c sums are DMA-transposed (in-stream, under the gather) into
    sentence-major cN[s,E] so the hops need no transposes at all:
    scores come out of PE directly as [sentence, batch], softmax runs
    unnormalized (exp + ones-matmul column sums + reciprocal), and the
    u update contracts over sentence partitions.
  - AllGather u across cores; vocab-sharded logits z = u @ A3^T with
    chunk pairs packed into all 128 PSUM partitions; log_softmax via
    AllReduce of exp-sums; final subtract/writeout pipelined.
"""

import numpy as np
import ml_dtypes

import concourse.bass as bass
import concourse.mybir as mybir
import concourse.tile as tile
from concourse import bacc
import concourse.bass_utils as bass_utils

F32 = mybir.dt.float32
AF = mybir.ActivationFunctionType
ALU = mybir.AluOpType
AX = mybir.AxisListType

NEG = -1e30


class Cfg:
    def __init__(self, ncore=8, B=64, S=50, J=64, QW=16, V=100000, E=128,
                 ucap=24576, gchunk=1024, use_bf16=True, z_f32=False,
                 g_fp8=False, z_fp8=False, rdma=False):
        self.ncore, self.B, self.S, self.J, self.QW = ncore, B, S, J, QW
        self.V, self.E, self.ucap, self.gchunk = V, E, ucap, gchunk
        self.use_bf16, self.z_f32 = use_bf16, z_f32
        self.g_fp8, self.z_fp8 = g_fp8, z_fp8
        self.rdma = rdma
        self.Bc = B // ncore
        self.NS = self.Bc * S              # sentences per core
        self.NX = self.NS * J              # story rows per core
        self.NQ = self.Bc * QW             # query rows per core
        assert self.NQ == 128 and self.NX % 128 == 0
        self.NPOS = self.NX + self.NQ
        self.xtiles = self.NX // 128
        self.SPT = 128 // J                # sentences per 128-row tile
        assert 128 % J == 0
        self.VSH = V // ncore
        sizes = []
        rem = self.NPOS
        while rem > 0:
            s = min(gchunk, rem)
            sizes.append(s)
            rem -= s
        assert all(s % 128 == 0 for s in sizes)
        self.gsizes = sizes
        self.nblk = (self.NS + 127) // 128  # 128-sentence blocks
        self.NSP = self.nblk * 128
        self.DT = mybir.dt.bfloat16 if use_bf16 else mybir.dt.float32
        self.npdt = ml_dtypes.bfloat16 if use_bf16 else np.float32
        # gather-table dtype (tables prescaled x16, weights /16 so the
        # PE reduction cancels the scale exactly)
        self.gdt = mybir.dt.float8e4 if g_fp8 else self.DT
        self.gnp = ml_dtypes.float8_e4m3 if g_fp8 else self.npdt
        assert not (z_f32 and z_fp8)
        self.zdt = F32 if z_f32 else (mybir.dt.float8e4 if z_fp8 else self.DT)
        self.znp = (np.float32 if z_f32 else
                    (ml_dtypes.float8_e4m3 if z_fp8 else self.npdt))

    def key(self):
        return (self.ncore, self.B, self.S, self.J, self.QW, self.V, self.E,
                self.ucap, self.gchunk, self.use_bf16, self.z_f32,
                self.g_fp8, self.z_fp8, self.rdma)


def build_module(cfg):
    c = cfg
    E, NS, Bc, VSH, S = c.E, c.NS, c.Bc, c.VSH, c.S
    NSP, nblk = c.NSP, c.nblk
    DT = c.DT
    nc = bacc.Bacc("TRN2", target_bir_lowering=False, debug=False,
                   num_devices=c.ncore, num_swdge_queues=4)

    GDT = c.gdt
    t_tab = nc.dram_tensor("tabc", [c.ucap, 4 * E], GDT, kind="ExternalInput")
    nch = len(c.gsizes)
    t_idx = nc.dram_tensor("idx", [nch, 128, c.gchunk // 16], mybir.dt.int16,
                           kind="ExternalInput")
    t_a3t = nc.dram_tensor("a3t", [E, VSH], c.zdt, kind="ExternalInput")
    t_wab = nc.dram_tensor("wab", [128, 2 * c.SPT], GDT, kind="ExternalInput")
    t_wabc = nc.dram_tensor("wabc", [128, 3 * c.SPT], GDT,
                            kind="ExternalInput")
    t_wc = nc.dram_tensor("wc", [128, c.SPT], GDT, kind="ExternalInput")
    t_wq = nc.dram_tensor("wq", [128, Bc], GDT, kind="ExternalInput")
    t_tatn = nc.dram_tensor("tatn", [E, NS], F32, kind="ExternalInput")
    t_tctn = nc.dram_tensor("tctn", [E, NS], F32, kind="ExternalInput")
    t_kp = nc.dram_tensor("kp", [E, 1], F32, kind="ExternalInput")
    t_mask2 = nc.dram_tensor("mask2", [128, nblk * Bc], F32,
                             kind="ExternalInput")
    t_ones = nc.dram_tensor("onesc", [128, 1], DT, kind="ExternalInput")
    t_ones1 = nc.dram_tensor("ones1", [1, 128], F32, kind="ExternalInput")

    ODT = DT if c.use_bf16 else F32
    t_o = nc.dram_tensor("o", [c.B, VSH], ODT, kind="ExternalOutput")
    t_du = nc.dram_tensor("du", [E, c.B], F32, kind="ExternalOutput")

    if c.rdma:
        # Raw (non-pool) SBUF receive buffers: written by REMOTE cores'
        # broadcasts, so Tile must not dep-track them — the explicit
        # wait_ge on the remote sem is the only correct gate.
        uall_t = nc.alloc_sbuf_tensor("uall", [E, c.ncore, Bc], F32)
        sall_t = nc.alloc_sbuf_tensor("sall", [128, c.ncore], F32)
        rsemU = nc.alloc_semaphore("rsemU")
        lsemU = nc.alloc_semaphore("lsemU")
        rsemS = nc.alloc_semaphore("rsemS")
        lsemS = nc.alloc_semaphore("lsemS")

    with tile.TileContext(nc) as tc:
        with tc.tile_pool(name="const", bufs=1) as cpool, \
             tc.tile_pool(name="gp", bufs=8) as gpool, \
             tc.tile_pool(name="wk", bufs=1) as wk, \
             tc.tile_pool(name="hp", bufs=2) as hp, \
             tc.tile_pool(name="big", bufs=1) as big, \
             tc.tile_pool(name="psG", bufs=3, space="PSUM") as psG, \
             tc.tile_pool(name="psH", bufs=1, space="PSUM") as psH, \
             tc.tile_pool(name="psZ", bufs=3, space="PSUM") as psZ, \
             tc.tile_pool(name="dram", bufs=1, space="DRAM") as dram:

            # ---- constant loads -------------------------------------------
            # Small consts first (the PE weights gate the whole reduction
            # pipeline), then the indices as ONE strided DMA; the big a3t
            # load goes on the scalar HWDGE queue out of the way.
            a3t = big.tile([E, VSH], c.zdt)
            nc.scalar.dma_start(out=a3t[:], in_=t_a3t.ap())
            wab = cpool.tile([128, 2 * c.SPT], GDT)
            nc.sync.dma_start(out=wab[:], in_=t_wab.ap())
            wabc = cpool.tile([128, 3 * c.SPT], GDT)
            nc.sync.dma_start(out=wabc[:], in_=t_wabc.ap())
            wc_t = cpool.tile([128, c.SPT], GDT)
            nc.sync.dma_start(out=wc_t[:], in_=t_wc.ap())
            wq = cpool.tile([128, Bc], GDT)
            nc.sync.dma_start(out=wq[:], in_=t_wq.ap())
            tatn = cpool.tile([E, NS], F32)
            nc.sync.dma_start(out=tatn[:], in_=t_tatn.ap())
            tctn = cpool.tile([E, NS], F32)
            nc.sync.dma_start(out=tctn[:], in_=t_tctn.ap())
            kp = cpool.tile([E, 1], F32)
            nc.sync.dma_start(out=kp[:], in_=t_kp.ap())
            mask2 = cpool.tile([128, nblk * Bc], F32)
            nc.sync.dma_start(out=mask2[:], in_=t_mask2.ap())
            onesc = cpool.tile([128, 1], DT)
            nc.sync.dma_start(out=onesc[:], in_=t_ones.ap())
            ones1 = cpool.tile([1, 128], F32)
            nc.sync.dma_start(out=ones1[:], in_=t_ones1.ap())
            idxs = cpool.tile([128, nch, c.gchunk // 16], mybir.dt.int16)
            nc.sync.dma_start(out=idxs[:],
                              in_=t_idx.ap().rearrange("g p w -> p g w"))

            # warm the Exp activation table during the gather phase
            dume = wk.tile([1, 1], F32, tag="dume")
            nc.scalar.activation(out=dume[:], in_=kp[:1, :1], func=AF.Exp)

            # ---- persistent work tiles ------------------------------------
            mT = [wk.tile([E, NSP], F32, tag=f"mT{h}", name=f"mT{h}")
                  for h in range(3)]
            cT = [wk.tile([E, NSP], DT, tag=f"cT{h}", name=f"cT{h}")
                  for h in range(3)]
            cN = [wk.tile([128, nblk, E], DT, tag=f"cN{h}", name=f"cN{h}")
                  for h in range(3)]
            uT = wk.tile([E, Bc], F32, tag="uT")
            if NSP > NS:
                for h in range(3):
                    nc.vector.memset(mT[h][:, NS:NSP], 0.0)
                    nc.vector.memset(cT[h][:, NS:NSP], 0.0)

            # ---- gather + per-chunk reductions + combines -----------------
            tile_idx = 0
            done_blk = 0
            for g, gs in enumerate(c.gsizes):
                slots = gs // 128
                nxt = min(slots, c.xtiles - tile_idx)   # x-tiles this chunk
                has_q = (tile_idx + slots) > c.xtiles
                spg = nxt * c.SPT
                gs0 = tile_idx * c.SPT
                L0, L1, L2 = 0, 2 * spg, 5 * spg
                L3, Lq = 8 * spg, 9 * spg

                gt = gpool.tile([128, c.gchunk // 128, 4 * E], GDT, tag="g")
                nc.gpsimd.dma_gather(
                    out_ap=gt[:, :slots, :],
                    in_ap=t_tab.ap(),
                    idxs_ap=idxs[:, g, : gs // 16],
                    num_idxs=gs,
                    num_idxs_reg=gs,
                    elem_size=4 * E,
                    queue_num=g % 4,
                )
                Pg = psG.tile([128, 512], F32, space="PSUM", tag="Pg")
                for sl in range(slots):
                    t = tile_idx
                    tile_idx += 1
                    if t < c.xtiles:
                        ls0 = (t * c.SPT) - gs0
                        G0 = gt[:, sl, 0 * E:1 * E]
                        G1 = gt[:, sl, 1 * E:2 * E]
                        G2 = gt[:, sl, 2 * E:3 * E]
                        G3 = gt[:, sl, 3 * E:4 * E]
                        nc.tensor.matmul(
                            out=Pg[:, L0 + 2 * ls0: L0 + 2 * ls0 + 2 * c.SPT],
                            lhsT=G0, rhs=wab[:], start=True, stop=True)
                        nc.tensor.matmul(
                            out=Pg[:, L1 + 3 * ls0: L1 + 3 * ls0 + 3 * c.SPT],
                            lhsT=G1, rhs=wabc[:], start=True, stop=True)
                        nc.tensor.matmul(
                            out=Pg[:, L2 + 3 * ls0: L2 + 3 * ls0 + 3 * c.SPT],
                            lhsT=G2, rhs=wabc[:], start=True, stop=True)
                        nc.tensor.matmul(
                            out=Pg[:, L3 + ls0: L3 + ls0 + c.SPT],
                            lhsT=G3, rhs=wc_t[:], start=True, stop=True)
                    else:
                        nc.tensor.matmul(
                            out=Pg[:, Lq: Lq + Bc],
                            lhsT=gt[:, sl, 0 * E:1 * E], rhs=wq[:],
                            start=True, stop=True)

                # per-chunk combines (psum -> sbuf slices, encodings folded)
                pap = Pg[:]
                pdim = pap.ap[0]

                def pv(base, gw, off, n=spg):
                    return bass.AP(pap.tensor, pap.offset + base + off,
                                   [pdim, (gw, n)])

                if spg > 0:
                    for h, (base, gw) in enumerate(
                            [(L0, 2), (L1, 3), (L2, 3)]):
                        msl = mT[h][:, gs0:gs0 + spg]
                        nc.vector.tensor_scalar(
                            out=msl, in0=pv(base, gw, 1), scalar1=kp[:],
                            scalar2=None, op0=ALU.mult)
                        nc.vector.tensor_tensor(
                            out=msl, in0=msl, in1=pv(base, gw, 0),
                            op=ALU.add)
                        nc.vector.tensor_tensor(
                            out=msl, in0=msl, in1=tatn[:, gs0:gs0 + spg],
                            op=ALU.add)
                    for h, (base, gw, off) in enumerate(
                            [(L1, 3, 2), (L2, 3, 2), (L3, 1, 0)]):
                        csl = cT[h][:, gs0:gs0 + spg]
                        nc.vector.tensor_tensor(
                            out=csl, in0=pv(base, gw, off),
                            in1=tctn[:, gs0:gs0 + spg], op=ALU.add)
                if has_q:
                    nc.vector.tensor_copy(uT[:], Pg[:, Lq: Lq + Bc])

                # DMA-transpose completed 128-sentence blocks of c into
                # sentence-major cN while the gather stream continues.
                end = gs0 + spg
                while done_blk < nblk and end >= min((done_blk + 1) * 128, NS):
                    b = done_blk
                    for h in range(3):
                        # scalar HWDGE queue: its inline waits must not block
                        # the sync queue that feeds everything else
                        nc.scalar.dma_start(
                            out=cN[h][:, b, :],
                            in_=cT[h][:, 128 * b:128 * (b + 1)],
                            transpose=True)
                    done_blk += 1

            # ---- hops (transpose-free) ------------------------------------
            for h in range(3):
                S_ps = psH.tile([128, (nblk + 1) * Bc], F32, space="PSUM",
                                tag="sc")
                for k in range(nblk):
                    nc.tensor.matmul(out=S_ps[:, k * Bc:(k + 1) * Bc],
                                     lhsT=mT[h][:, 128 * k:128 * (k + 1)],
                                     rhs=uT[:], start=True, stop=True)
                ex = hp.tile([128, nblk * Bc], F32, tag="ex")
                nc.vector.tensor_tensor(out=ex[:], in0=S_ps[:, :nblk * Bc],
                                        in1=mask2[:], op=ALU.add)
                P = hp.tile([128, nblk, Bc], DT, tag="P")
                nc.scalar.activation(out=P[:].rearrange("p k b -> p (k b)"),
                                     in_=ex[:], func=AF.Exp)
                U_ps = psH.tile([E, 2 * Bc], F32, space="PSUM", tag="up")
                for k in range(nblk):
                    nc.tensor.matmul(out=U_ps[:, :Bc], lhsT=cN[h][:, k, :],
                                     rhs=P[:, k, :], start=(k == 0),
                                     stop=(k == nblk - 1))
                for k in range(nblk):
                    nc.tensor.matmul(
                        out=S_ps[:1, nblk * Bc:(nblk + 1) * Bc],
                        lhsT=onesc[:], rhs=P[:, k, :], start=(k == 0),
                        stop=(k == nblk - 1))
                rz1 = hp.tile([1, Bc], F32, tag="rz1")
                nc.vector.reciprocal(
                    out=rz1[:], in_=S_ps[:1, nblk * Bc:(nblk + 1) * Bc])
                nc.tensor.matmul(out=U_ps[:, Bc:2 * Bc], lhsT=ones1[:],
                                 rhs=rz1[:], start=True, stop=True)
                rzc = hp.tile([E, Bc], F32, tag="rzc")
                nc.vector.tensor_copy(rzc[:], U_ps[:, Bc:2 * Bc])
                un = wk.tile([E, Bc], F32, tag=f"uT{h + 1}")
                nc.vector.tensor_tensor(out=un[:], in0=U_ps[:, :Bc],
                                        in1=rzc[:], op=ALU.mult)
                nc.vector.tensor_tensor(out=un[:], in0=un[:], in1=uT[:],
                                        op=ALU.add)
                uT = un

            # ---- AllGather u ----------------------------------------------
            uz = wk.tile([E, c.B], c.zdt, tag="uz")
            rdma_fixups = []
            if c.rdma:
                # direct 1-hop allgather: each core broadcasts its u into
                # its own rank slot on every peer (incl. self); every
                # receiver's rsemU reaches 16 when all 8 frames landed
                rank = nc.gpsimd.partition_id()
                for r in range(c.ncore):
                    with tc.If(rank == r):
                        nc.gpsimd.remote_dma_broadcast(
                            out_ap=uall_t.ap()[:, r, :],
                            in_ap=uT[:],
                            remote_sem=rsemU,
                            local_sem=lsemU,
                            rdests=[(0, k) for k in range(c.ncore)],
                        )
                        nc.gpsimd.trigger_dma(count=1)
                # zero token written after the hops gives the consumers a
                # tracked dep (keeps scheduler order); the remote-arrival
                # gate (rsemU >= 16) is appended post-schedule.
                zmask = wk.tile([E, 1], F32, tag="zmask")
                nc.vector.tensor_scalar(out=zmask[:], in0=uT[:, :1],
                                        scalar1=0.0, scalar2=None,
                                        op0=ALU.mult)
                uview = uall_t.ap().rearrange("e c b -> e (c b)")
                zma = zmask[:]
                zb = bass.AP(zma.tensor, zma.offset, [zma.ap[0], (0, c.B)])
                i_uz = nc.vector.tensor_tensor(out=uz[:], in0=uview, in1=zb,
                                               op=ALU.add)
                duf = wk.tile([E, c.B], F32, tag="duf")
                i_duf = nc.vector.tensor_tensor(out=duf[:], in0=uview,
                                                in1=zb, op=ALU.add)
                nc.sync.dma_start(out=t_du.ap(), in_=duf[:])
                rdma_fixups += [(i_uz, rsemU), (i_duf, rsemU)]
            else:
                ub_in = dram.tile([E, Bc], F32)
                ub_out = dram.tile([c.ncore * E, Bc], F32)
                nc.sync.dma_start(out=ub_in[:], in_=uT[:])
                nc.gpsimd.collective_compute(
                    "AllGather", ALU.bypass,
                    replica_groups=[list(range(c.ncore))],
                    ins=[ub_in.opt()], outs=[ub_out.opt()],
                )
                uTf = wk.tile([E, c.ncore, Bc], F32, tag="uTf")
                src = bass.AP(ub_out[:].tensor, ub_out[:].offset,
                              [(Bc, E), (E * Bc, c.ncore), (1, Bc)])
                nc.sync.dma_start(out=uTf[:], in_=src)
                nc.sync.dma_start(out=t_du.ap(),
                                  in_=uTf[:].rearrange("e c b -> e (c b)"))
                nc.vector.tensor_copy(uz[:],
                                      uTf[:].rearrange("e c b -> e (c b)"))

            # ---- logits + log_softmax (chunk pairs on 128 partitions) -----
            npar = VSH // 1024
            rem = VSH - npar * 1024
            assert rem <= 512, (VSH, npar, rem)
            nzc2 = npar + (1 if rem else 0)
            zW = 512 * npar + rem
            # store exp(z) (bf16) instead of z: the final pass recovers
            # log-probs exactly as Ln(exp(z) / sum) with a per-partition
            # reciprocal scale — no DVE copy in this loop at all
            escb = big.tile([128, zW], DT)
            sums2 = wk.tile([128, max(nzc2, 1)], F32, tag="sums2")
            for i in range(npar):
                zps = psZ.tile([128, 512], F32, space="PSUM", tag="zps")
                nc.tensor.matmul(out=zps[:c.B, :], lhsT=uz[:],
                                 rhs=a3t[:, 1024 * i:1024 * i + 512],
                                 start=True, stop=True)
                nc.tensor.matmul(out=zps[c.B:128, :], lhsT=uz[:],
                                 rhs=a3t[:, 1024 * i + 512:1024 * (i + 1)],
                                 start=True, stop=True)
                nc.scalar.activation(out=escb[:, 512 * i:512 * (i + 1)],
                                     in_=zps[:], func=AF.Exp,
                                     accum_out=sums2[:, i:i + 1])
            if rem:
                zps = psZ.tile([128, 512], F32, space="PSUM", tag="zps")
                nc.tensor.matmul(out=zps[:c.B, :rem], lhsT=uz[:],
                                 rhs=a3t[:, npar * 1024:VSH],
                                 start=True, stop=True)
                nc.scalar.activation(out=escb[:c.B, 512 * npar:zW],
                                     in_=zps[:c.B, :rem], func=AF.Exp,
                                     accum_out=sums2[:c.B, npar:npar + 1])

            slcio = wk.tile([128, 1], F32, tag="slcio")
            nc.vector.tensor_reduce(out=slcio[:c.B, :],
                                    in_=sums2[:c.B, :nzc2], axis=AX.X,
                                    op=ALU.add)
            if npar > 0:
                nc.vector.tensor_reduce(out=slcio[c.B:128, :],
                                        in_=sums2[c.B:128, :npar], axis=AX.X,
                                        op=ALU.add)
            else:
                nc.vector.memset(slcio[c.B:128, :], 0.0)
            if c.rdma:
                for r in range(c.ncore):
                    with tc.If(rank == r):
                        nc.gpsimd.remote_dma_broadcast(
                            out_ap=sall_t.ap()[:, r:r + 1],
                            in_ap=slcio[:],
                            remote_sem=rsemS,
                            local_sem=lsemS,
                            rdests=[(0, k) for k in range(c.ncore)],
                        )
                        nc.gpsimd.trigger_dma(count=1)
                smask = wk.tile([128, 1], F32, tag="smask")
                nc.vector.tensor_scalar(out=smask[:], in0=slcio[:],
                                        scalar1=0.0, scalar2=None,
                                        op0=ALU.mult)
                sma = smask[:]
                sbb = bass.AP(sma.tensor, sma.offset,
                              [sma.ap[0], (0, c.ncore)])
                stmp = wk.tile([128, c.ncore], F32, tag="stmp")
                i_sm = nc.vector.tensor_tensor(out=stmp[:], in0=sall_t.ap(),
                                               in1=sbb, op=ALU.add)
                rdma_fixups.append((i_sm, rsemS))
                red = wk.tile([128, 1], F32, tag="red")
                nc.vector.tensor_reduce(out=red[:], in_=stmp[:],
                                        axis=AX.X, op=ALU.add)
                # swap partition halves so every lane has top+bottom sums
                redsw = wk.tile([128, 1], F32, tag="redsw")
                nc.sync.dma_start(out=redsw[:c.B, :], in_=red[c.B:128, :])
                nc.sync.dma_start(out=redsw[c.B:128, :], in_=red[:c.B, :])
                stF = wk.tile([128, 1], F32, tag="stF")
                nc.vector.tensor_tensor(out=stF[:], in0=red[:], in1=redsw[:],
                                        op=ALU.add)
            else:
                sb_in = dram.tile([128, 1], F32)
                sb_out = dram.tile([128, 1], F32)
                nc.sync.dma_start(out=sb_in[:], in_=slcio[:])
                nc.gpsimd.collective_compute(
                    "AllReduce", ALU.add,
                    replica_groups=[list(range(c.ncore))],
                    ins=[sb_in.opt()], outs=[sb_out.opt()],
                )
                stA = wk.tile([128, 1], F32, tag="stA")
                stB = wk.tile([128, 1], F32, tag="stB")
                so = sb_out[:]
                nc.sync.dma_start(out=stA[:c.B, :], in_=so[:c.B])
                nc.sync.dma_start(out=stA[c.B:128, :], in_=so[:c.B])
                nc.sync.dma_start(out=stB[:c.B, :], in_=so[c.B:128])
                nc.sync.dma_start(out=stB[c.B:128, :], in_=so[c.B:128])
                stF = wk.tile([128, 1], F32, tag="stF")
                nc.vector.tensor_tensor(out=stF[:], in0=stA[:], in1=stB[:],
                                        op=ALU.add)
            rstF = wk.tile([128, 1], F32, tag="rstF")
            nc.vector.reciprocal(out=rstF[:], in_=stF[:])

            # out = Ln(exp(z) * (1/sum)) = z - lse, streamed out in slices
            obuf = big.tile([128, zW], ODT)
            PSL = 4
            i0 = 0
            while i0 < npar:
                i1 = min(i0 + PSL, npar)
                nc.scalar.activation(
                    out=obuf[:, 512 * i0:512 * i1],
                    in_=escb[:, 512 * i0:512 * i1],
                    func=AF.Ln, scale=rstF[:])
                dst_e = bass.AP(t_o.ap().tensor, 1024 * i0,
                                [(VSH, c.B), (1024, i1 - i0), (1, 512)])
                nc.sync.dma_start(
                    out=dst_e,
                    in_=obuf[:c.B, 512 * i0:512 * i1].rearrange(
                        "b (i j) -> b i j", j=512))
                dst_o = bass.AP(t_o.ap().tensor, 1024 * i0 + 512,
                                [(VSH, c.B), (1024, i1 - i0), (1, 512)])
                nc.sync.dma_start(
                    out=dst_o,
                    in_=obuf[c.B:128, 512 * i0:512 * i1].rearrange(
                        "b (i j) -> b i j", j=512))
                i0 = i1
            if rem:
                nc.scalar.activation(
                    out=obuf[:c.B, 512 * npar:zW],
                    in_=escb[:c.B, 512 * npar:zW],
                    func=AF.Ln, scale=rstF[:c.B, :])
                dst_r = bass.AP(t_o.ap().tensor, 1024 * npar,
                                [(VSH, c.B), (1, rem)])
                nc.sync.dma_start(out=dst_r, in_=obuf[:c.B, 512 * npar:zW])

    if c.rdma:
        # Attach the remote-arrival gates AFTER Tile scheduling: the
        # single-core scheduling sim cannot satisfy waits fed by peers'
        # remote DMAs (it would report a deadlock), but the hardware can.
        for inst, sem in rdma_fixups:
            target = inst.ins
            placed = False
            for blk in nc.main_func.blocks:
                for idx, i2 in enumerate(blk.instructions):
                    if i2 is target:
                        w = mybir.SyncWait(
                            sync_type="semaphore", id=sem.num,
                            wait_mode="sem-ge-imm", wait_value=16,
                            ant_name=sem.name)
                        ev = mybir.InstEventSemaphore(
                            name=nc.get_next_instruction_name(),
                            ins=[], outs=[])
                        ev.engine = target.engine
                        ev.sync_info = mybir.SyncInfo(on_wait=[w],
                                                      on_update=[])
                        nc.register_instruction(ev)
                        blk.instructions.insert(idx, ev)
                        placed = True
                        break
                if placed:
                    break
            assert placed, "rdma wait target instruction not found"

    nc.compile()
    return nc


def host_prep(cfg, x, q, A, TA, TC):
    c = cfg
    E, J, S = c.E, c.J, c.S
    x = np.asarray(x).astype(np.int64)
    q = np.asarray(q).astype(np.int64)
    A = np.asarray(A, dtype=np.float32)
    TA = np.asarray(TA, dtype=np.float32)
    TC = np.asarray(TC, dtype=np.float32)

    tabI = np.ascontiguousarray(A.transpose(1, 0, 2).reshape(c.V, 4 * E))
    wscale = 1.0
    if c.g_fp8:
        # prescale x16 lifts N(0,0.1) values out of e4m3 denormal range;
        # the /16 on the one-hot PE weights cancels it exactly in the matmul
        tabI = (tabI * 16.0).astype(c.gnp)
        wscale = 1.0 / 16.0
    else:
        tabI = tabI.astype(c.npdt)
    a3tF = np.ascontiguousarray(A[3].T)  # [E, V] f32
    if c.z_fp8:
        a3tF = a3tF * 16.0

    j = np.arange(1, J + 1, dtype=np.float32)
    av = 1.0 - j / J
    bv = 2.0 * j / J - 1.0
    sp = np.arange(128) // J
    jj = np.arange(128) % J
    wab = np.zeros((128, 2 * c.SPT), np.float32)
    wabc = np.zeros((128, 3 * c.SPT), np.float32)
    wc = np.zeros((128, c.SPT), np.float32)
    for p in range(128):
        wab[p, 2 * sp[p] + 0] = av[jj[p]]
        wab[p, 2 * sp[p] + 1] = bv[jj[p]]
        wabc[p, 3 * sp[p] + 0] = av[jj[p]]
        wabc[p, 3 * sp[p] + 1] = bv[jj[p]]
        wabc[p, 3 * sp[p] + 2] = 1.0
        wc[p, sp[p]] = 1.0
    wq = np.zeros((128, c.Bc), np.float32)
    for p in range(128):
        wq[p, p // c.QW] = 1.0

    tat = np.ascontiguousarray(TA[0, :S, :].T)   # [E, S]
    tct = np.ascontiguousarray(TC[0, :S, :].T)
    tatn = np.tile(tat, (1, c.Bc))               # [E, NS] batch-major
    tctn = np.tile(tct, (1, c.Bc))
    kp = ((np.arange(E, dtype=np.float32) + 1.0) / E).reshape(E, 1)
    mask2 = np.full((128, c.nblk * c.Bc), NEG, np.float32)
    for k in range(c.nblk):
        for p in range(128):
            s = k * 128 + p
            if s < c.NS:
                mask2[p, k * c.Bc + s // S] = 0.0
    onesc = np.ones((128, 1), np.float32)
    ones1 = np.ones((1, 128), np.float32)

    common = {
        "wab": (wab * wscale).astype(c.gnp),
        "wabc": (wabc * wscale).astype(c.gnp),
        "wc": (wc * wscale).astype(c.gnp),
        "wq": (wq * wscale).astype(c.gnp),
        "tatn": np.ascontiguousarray(tatn),
        "tctn": np.ascontiguousarray(tctn),
        "kp": kp, "mask2": mask2,
        "onesc": onesc.astype(c.npdt), "ones1": ones1,
    }

    nch = len(c.gsizes)
    in_maps = []
    for cc in range(c.ncore):
        xc = x[cc * c.Bc:(cc + 1) * c.Bc].reshape(-1)
        qc = q[cc * c.Bc:(cc + 1) * c.Bc].reshape(-1)
        xq = np.concatenate([xc, qc])
        uniq, rel = np.unique(xq, return_inverse=True)
        assert len(uniq) <= c.ucap, (len(uniq), c.ucap)
        tabc = np.zeros((c.ucap, 4 * E), c.gnp)
        tabc[:len(uniq)] = tabI[uniq]
        rel = rel.astype(np.int16)
        idx = np.zeros((nch, 128, c.gchunk // 16), np.int16)
        off = 0
        for g, gs in enumerate(c.gsizes):
            v = rel[off:off + gs]
            off += gs
            wrapped = v.reshape(-1, 16).T
            idx[g, :, : gs // 16] = np.tile(wrapped, (8, 1))
        a3c = np.ascontiguousarray(
            a3tF[:, cc * c.VSH:(cc + 1) * c.VSH]).astype(c.znp)
        m = dict(common)
        m.update({"tabc": tabc, "idx": idx, "a3t": a3c})
        in_maps.append(m)
    return in_maps


_CACHE = {}


def _get_module(cfg):
    k = cfg.key()
    if k not in _CACHE:
        _CACHE[k] = build_module(cfg)
    return _CACHE[k]


def run(cfg, inputs, trace=False):
    nc = _get_module(cfg)
    in_maps = host_prep(cfg, inputs["x"], inputs["q"], inputs["A"],
                        inputs["TA"], inputs["TC"])
    res = bass_utils.run_bass_kernel_spmd(
        nc, in_maps, core_ids=list(range(cfg.ncore)), trace=trace)
    out = np.concatenate(
        [np.asarray(res.results[cc]["o"]).astype(np.float32)
         for cc in range(cfg.ncore)], axis=1)
    return out, res


def kernel(**inputs) -> np.ndarray:
    cfg = Cfg()
    out, _ = run(cfg, inputs, trace=False)
    return out


# revision 47
# speedup vs baseline: 1.1784x; 1.1784x over previous
"""MemNN (embedding_lookup) Trainium2 Bass kernel.

Strategy (8 NeuronCores, one NEFF, SPMD):
  - Data-parallel hops: batch dim sharded 8 ways (8 batches/core).
  - Host packs the 4 embedding tables interleaved per vocab row
    ([A0|A1|A2|A3][v], bf16) and, per core, compacts it to the core's
    unique vocab rows so indices fit dma_gather's int16 (~22.6K < 32767).
  - dma_gather streams all (story + query) embedding rows across the
    4 SWDGE queues (round-robin) so descriptor generation runs on all
    4 Q7 core-pairs concurrently; PE matmuls with fixed block weights
    reduce each 128-row tile into per-sentence partial sums (PSUM).
    Position encoding is rank-2 separable:
        pe[j,d] = a(j) + b(j) * k'(d),  a=1-j/J, b=2j/J-1, k'=d/D
    so m = S_a + k' * S_b needs only two weighted sums per sentence.
    Temporal encodings are folded into the per-chunk combines.
  - c sums are DMA-transposed (in-stream, under the gather) into
    sentence-major cN[s,E] so the hops need no transposes at all:
    scores come out of PE directly as [sentence, batch], softmax runs
    unnormalized (exp + ones-matmul column sums + reciprocal), and the
    u update contracts over sentence partitions.
  - AllGather u across cores; vocab-sharded logits z = u @ A3^T with
    chunk pairs packed into all 128 PSUM partitions; log_softmax via
    AllReduce of exp-sums; final subtract/writeout pipelined.
"""

import numpy as np
import ml_dtypes

import concourse.bass as bass
import concourse.mybir as mybir
import concourse.tile as tile
from concourse import bacc
import concourse.bass_utils as bass_utils

F32 = mybir.dt.float32
AF = mybir.ActivationFunctionType
ALU = mybir.AluOpType
AX = mybir.AxisListType

NEG = -1e30


class Cfg:
    def __init__(self, ncore=8, B=64, S=50, J=64, QW=16, V=100000, E=128,
                 ucap=24576, gchunk=1024, use_bf16=True, z_f32=False,
                 g_fp8=False, z_fp8=False, rdma=False):
        self.ncore, self.B, self.S, self.J, self.QW = ncore, B, S, J, QW
        self.V, self.E, self.ucap, self.gchunk = V, E, ucap, gchunk
        self.use_bf16, self.z_f32 = use_bf16, z_f32
        self.g_fp8, self.z_fp8 = g_fp8, z_fp8
        self.rdma = rdma
        self.Bc = B // ncore
        self.NS = self.Bc * S              # sentences per core
        self.NX = self.NS * J              # story rows per core
        self.NQ = self.Bc * QW             # query rows per core
        assert self.NQ == 128 and self.NX % 128 == 0
        self.NPOS = self.NX + self.NQ
        self.xtiles = self.NX // 128
        self.SPT = 128 // J                # sentences per 128-row tile
        assert 128 % J == 0
        self.VSH = V // ncore
        sizes = []
        rem = self.NPOS
        while rem > 0:
            s = min(gchunk, rem)
            sizes.append(s)
            rem -= s
        assert all(s % 128 == 0 for s in sizes)
        self.gsizes = sizes
        self.nblk = (self.NS + 127) // 128  # 128-sentence blocks
        self.NSP = self.nblk * 128
        self.DT = mybir.dt.bfloat16 if use_bf16 else mybir.dt.float32
        self.npdt = ml_dtypes.bfloat16 if use_bf16 else np.float32
        # gather-table dtype (tables prescaled x16, weights /16 so the
        # PE reduction cancels the scale exactly)
        self.gdt = mybir.dt.float8e4 if g_fp8 else self.DT
        self.gnp = ml_dtypes.float8_e4m3 if g_fp8 else self.npdt
        assert not (z_f32 and z_fp8)
        self.zdt = F32 if z_f32 else (mybir.dt.float8e4 if z_fp8 else self.DT)
        self.znp = (np.float32 if z_f32 else
                    (ml_dtypes.float8_e4m3 if z_fp8 else self.npdt))

    def key(self):
        return (self.ncore, self.B, self.S, self.J, self.QW, self.V, self.E,
                self.ucap, self.gchunk, self.use_bf16, self.z_f32,
                self.g_fp8, self.z_fp8, self.rdma)


def build_module(cfg):
    c = cfg
    E, NS, Bc, VSH, S = c.E, c.NS, c.Bc, c.VSH, c.S
    NSP, nblk = c.NSP, c.nblk
    DT = c.DT
    nc = bacc.Bacc("TRN2", target_bir_lowering=False, debug=False,
                   num_devices=c.ncore, num_swdge_queues=4)

    GDT = c.gdt
    t_tab = nc.dram_tensor("tabc", [c.ucap, 4 * E], GDT, kind="ExternalInput")
    nch = len(c.gsizes)
    t_idx = nc.dram_tensor("idx", [nch, 128, c.gchunk // 16], mybir.dt.int16,
                           kind="ExternalInput")
    t_a3t = nc.dram_tensor("a3t", [E, VSH], c.zdt, kind="ExternalInput")
    t_wab = nc.dram_tensor("wab", [128, 2 * c.SPT], GDT, kind="ExternalInput")
    t_wabc = nc.dram_tensor("wabc", [128, 3 * c.SPT], GDT,
                            kind="ExternalInput")
    t_wc = nc.dram_tensor("wc", [128, c.SPT], GDT, kind="ExternalInput")
    t_wq = nc.dram_tensor("wq", [128, Bc], GDT, kind="ExternalInput")
    t_tatn = nc.dram_tensor("tatn", [E, NS], F32, kind="ExternalInput")
    t_tctn = nc.dram_tensor("tctn", [E, NS], F32, kind="ExternalInput")
    t_kp = nc.dram_tensor("kp", [E, 1], F32, kind="ExternalInput")
    t_mask2 = nc.dram_tensor("mask2", [128, nblk * Bc], F32,
                             kind="ExternalInput")
    t_ones = nc.dram_tensor("onesc", [128, 1], DT, kind="ExternalInput")
    t_ones1 = nc.dram_tensor("ones1", [1, 128], F32, kind="ExternalInput")

    ODT = DT if c.use_bf16 else F32
    t_o = nc.dram_tensor("o", [c.B, VSH], ODT, kind="ExternalOutput")
    t_du = nc.dram_tensor("du", [E, c.B], F32, kind="ExternalOutput")

    if c.rdma:
        # Raw (non-pool) SBUF receive buffers: written by REMOTE cores'
        # broadcasts, so Tile must not dep-track them — the explicit
        # wait_ge on the remote sem is the only correct gate.
        uall_t = nc.alloc_sbuf_tensor("uall", [E, c.ncore, Bc], F32)
        sall_t = nc.alloc_sbuf_tensor("sall", [128, c.ncore], F32)
        rsemU = nc.alloc_semaphore("rsemU")
        lsemU = nc.alloc_semaphore("lsemU")
        rsemS = nc.alloc_semaphore("rsemS")
        lsemS = nc.alloc_semaphore("lsemS")

    with tile.TileContext(nc) as tc:
        with tc.tile_pool(name="const", bufs=1) as cpool, \
             tc.tile_pool(name="gp", bufs=8) as gpool, \
             tc.tile_pool(name="wk", bufs=1) as wk, \
             tc.tile_pool(name="hp", bufs=2) as hp, \
             tc.tile_pool(name="big", bufs=1) as big, \
             tc.tile_pool(name="psG", bufs=3, space="PSUM") as psG, \
             tc.tile_pool(name="psH", bufs=1, space="PSUM") as psH, \
             tc.tile_pool(name="psZ", bufs=3, space="PSUM") as psZ, \
             tc.tile_pool(name="dram", bufs=1, space="DRAM") as dram:

            # ---- constant loads -------------------------------------------
            # Small consts first (the PE weights gate the whole reduction
            # pipeline), then the indices as ONE strided DMA; the big a3t
            # load goes on the scalar HWDGE queue out of the way.
            a3t = big.tile([E, VSH], c.zdt)
            nc.scalar.dma_start(out=a3t[:], in_=t_a3t.ap())
            wab = cpool.tile([128, 2 * c.SPT], GDT)
            nc.sync.dma_start(out=wab[:], in_=t_wab.ap())
            wabc = cpool.tile([128, 3 * c.SPT], GDT)
            nc.sync.dma_start(out=wabc[:], in_=t_wabc.ap())
            wc_t = cpool.tile([128, c.SPT], GDT)
            nc.sync.dma_start(out=wc_t[:], in_=t_wc.ap())
            wq = cpool.tile([128, Bc], GDT)
            nc.sync.dma_start(out=wq[:], in_=t_wq.ap())
            tatn = cpool.tile([E, NS], F32)
            nc.sync.dma_start(out=tatn[:], in_=t_tatn.ap())
            tctn = cpool.tile([E, NS], F32)
            nc.sync.dma_start(out=tctn[:], in_=t_tctn.ap())
            kp = cpool.tile([E, 1], F32)
            nc.sync.dma_start(out=kp[:], in_=t_kp.ap())
            mask2 = cpool.tile([128, nblk * Bc], F32)
            nc.sync.dma_start(out=mask2[:], in_=t_mask2.ap())
            onesc = cpool.tile([128, 1], DT)
            nc.sync.dma_start(out=onesc[:], in_=t_ones.ap())
            ones1 = cpool.tile([1, 128], F32)
            nc.sync.dma_start(out=ones1[:], in_=t_ones1.ap())
            idxs = cpool.tile([128, nch, c.gchunk // 16], mybir.dt.int16)
            nc.sync.dma_start(out=idxs[:],
                              in_=t_idx.ap().rearrange("g p w -> p g w"))

            # warm the Exp activation table during the gather phase
            dume = wk.tile([1, 1], F32, tag="dume")
            nc.scalar.activation(out=dume[:], in_=kp[:1, :1], func=AF.Exp)

            # ---- persistent work tiles ------------------------------------
            mT = [wk.tile([E, NSP], F32, tag=f"mT{h}", name=f"mT{h}")
                  for h in range(3)]
            cT = [wk.tile([E, NSP], DT, tag=f"cT{h}", name=f"cT{h}")
                  for h in range(3)]
            cN = [wk.tile([128, nblk, E], DT, tag=f"cN{h}", name=f"cN{h}")
                  for h in range(3)]
            uT = wk.tile([E, Bc], F32, tag="uT")
            if NSP > NS:
                for h in range(3):
                    nc.vector.memset(mT[h][:, NS:NSP], 0.0)
                    nc.vector.memset(cT[h][:, NS:NSP], 0.0)

            # ---- gather + per-chunk reductions + combines -----------------
            tile_idx = 0
            done_blk = 0
            for g, gs in enumerate(c.gsizes):
                slots = gs // 128
                nxt = min(slots, c.xtiles - tile_idx)   # x-tiles this chunk
                has_q = (tile_idx + slots) > c.xtiles
                spg = nxt * c.SPT
                gs0 = tile_idx * c.SPT
                L0, L1, L2 = 0, 2 * spg, 5 * spg
                L3, Lq = 8 * spg, 9 * spg

                gt = gpool.tile([128, c.gchunk // 128, 4 * E], GDT, tag="g")
                nc.gpsimd.dma_gather(
                    out_ap=gt[:, :slots, :],
                    in_ap=t_tab.ap(),
                    idxs_ap=idxs[:, g, : gs // 16],
                    num_idxs=gs,
                    num_idxs_reg=gs,
                    elem_size=4 * E,
                    queue_num=g % 4,
                )
                Pg = psG.tile([128, 512], F32, space="PSUM", tag="Pg")
                for sl in range(slots):
                    t = tile_idx
                    tile_idx += 1
                    if t < c.xtiles:
                        ls0 = (t * c.SPT) - gs0
                        G0 = gt[:, sl, 0 * E:1 * E]
                        G1 = gt[:, sl, 1 * E:2 * E]
                        G2 = gt[:, sl, 2 * E:3 * E]
                        G3 = gt[:, sl, 3 * E:4 * E]
                        nc.tensor.matmul(
                            out=Pg[:, L0 + 2 * ls0: L0 + 2 * ls0 + 2 * c.SPT],
                            lhsT=G0, rhs=wab[:], start=True, stop=True)
                        nc.tensor.matmul(
                            out=Pg[:, L1 + 3 * ls0: L1 + 3 * ls0 + 3 * c.SPT],
                            lhsT=G1, rhs=wabc[:], start=True, stop=True)
                        nc.tensor.matmul(
                            out=Pg[:, L2 + 3 * ls0: L2 + 3 * ls0 + 3 * c.SPT],
                            lhsT=G2, rhs=wabc[:], start=True, stop=True)
                        nc.tensor.matmul(
                            out=Pg[:, L3 + ls0: L3 + ls0 + c.SPT],
                            lhsT=G3, rhs=wc_t[:], start=True, stop=True)
                    else:
                        nc.tensor.matmul(
                            out=Pg[:, Lq: Lq + Bc],
                            lhsT=gt[:, sl, 0 * E:1 * E], rhs=wq[:],
                            start=True, stop=True)

                # per-chunk combines (psum -> sbuf slices, encodings folded)
                pap = Pg[:]
                pdim = pap.ap[0]

                def pv(base, gw, off, n=spg):
                    return bass.AP(pap.tensor, pap.offset + base + off,
                                   [pdim, (gw, n)])

                if spg > 0:
                    for h, (base, gw) in enumerate(
                            [(L0, 2), (L1, 3), (L2, 3)]):
                        msl = mT[h][:, gs0:gs0 + spg]
                        nc.vector.tensor_scalar(
                            out=msl, in0=pv(base, gw, 1), scalar1=kp[:],
                            scalar2=None, op0=ALU.mult)
                        nc.vector.tensor_tensor(
                            out=msl, in0=msl, in1=pv(base, gw, 0),
                            op=ALU.add)
                        nc.vector.tensor_tensor(
                            out=msl, in0=msl, in1=tatn[:, gs0:gs0 + spg],
                            op=ALU.add)
                    for h, (base, gw, off) in enumerate(
                            [(L1, 3, 2), (L2, 3, 2), (L3, 1, 0)]):
                        csl = cT[h][:, gs0:gs0 + spg]
                        nc.vector.tensor_tensor(
                            out=csl, in0=pv(base, gw, off),
                            in1=tctn[:, gs0:gs0 + spg], op=ALU.add)
                if has_q:
                    nc.vector.tensor_copy(uT[:], Pg[:, Lq: Lq + Bc])

                # DMA-transpose completed 128-sentence blocks of c into
                # sentence-major cN while the gather stream continues.
                end = gs0 + spg
                while done_blk < nblk and end >= min((done_blk + 1) * 128, NS):
                    b = done_blk
                    for h in range(3):
                        # scalar HWDGE queue: its inline waits must not block
                        # the sync queue that feeds everything else
                        nc.scalar.dma_start(
                            out=cN[h][:, b, :],
                            in_=cT[h][:, 128 * b:128 * (b + 1)],
                            transpose=True)
                    done_blk += 1

            # ---- hops (transpose-free) ------------------------------------
            for h in range(3):
                S_ps = psH.tile([128, (nblk + 1) * Bc], F32, space="PSUM",
                                tag="sc")
                for k in range(nblk):
                    nc.tensor.matmul(out=S_ps[:, k * Bc:(k + 1) * Bc],
                                     lhsT=mT[h][:, 128 * k:128 * (k + 1)],
                                     rhs=uT[:], start=True, stop=True)
                ex = hp.tile([128, nblk * Bc], F32, tag="ex")
                nc.vector.tensor_tensor(out=ex[:], in0=S_ps[:, :nblk * Bc],
                                        in1=mask2[:], op=ALU.add)
                P = hp.tile([128, nblk, Bc], DT, tag="P")
                nc.scalar.activation(out=P[:].rearrange("p k b -> p (k b)"),
                                     in_=ex[:], func=AF.Exp)
                U_ps = psH.tile([E, 2 * Bc], F32, space="PSUM", tag="up")
                for k in range(nblk):
                    nc.tensor.matmul(out=U_ps[:, :Bc], lhsT=cN[h][:, k, :],
                                     rhs=P[:, k, :], start=(k == 0),
                                     stop=(k == nblk - 1))
                for k in range(nblk):
                    nc.tensor.matmul(
                        out=S_ps[:1, nblk * Bc:(nblk + 1) * Bc],
                        lhsT=onesc[:], rhs=P[:, k, :], start=(k == 0),
                        stop=(k == nblk - 1))
                rz1 = hp.tile([1, Bc], F32, tag="rz1")
                nc.vector.reciprocal(
                    out=rz1[:], in_=S_ps[:1, nblk * Bc:(nblk + 1) * Bc])
                nc.tensor.matmul(out=U_ps[:, Bc:2 * Bc], lhsT=ones1[:],
                                 rhs=rz1[:], start=True, stop=True)
                rzc = hp.tile([E, Bc], F32, tag="rzc")
                nc.vector.tensor_copy(rzc[:], U_ps[:, Bc:2 * Bc])
                un = wk.tile([E, Bc], F32, tag=f"uT{h + 1}")
                nc.vector.tensor_tensor(out=un[:], in0=U_ps[:, :Bc],
                                        in1=rzc[:], op=ALU.mult)
                nc.vector.tensor_tensor(out=un[:], in0=un[:], in1=uT[:],
                                        op=ALU.add)
                uT = un

            # ---- AllGather u ----------------------------------------------
            uz = wk.tile([E, c.B], c.zdt, tag="uz")
            rdma_fixups = []
            if c.rdma:
                # direct 1-hop allgather: each core broadcasts its u into
                # its own rank slot on every peer (incl. self); every
                # receiver's rsemU reaches 16 when all 8 frames landed
                rank = nc.gpsimd.partition_id()
                for r in range(c.ncore):
                    with tc.If(rank == r):
                        nc.gpsimd.remote_dma_broadcast(
                            out_ap=uall_t.ap()[:, r, :],
                            in_ap=uT[:],
                            remote_sem=rsemU,
                            local_sem=lsemU,
                            rdests=[(0, k) for k in range(c.ncore)],
                        )
                        nc.gpsimd.trigger_dma(count=1)
                # zero token written after the hops gives the consumers a
                # tracked dep (keeps scheduler order); the remote-arrival
                # gate (rsemU >= 16) is appended post-schedule.
                zmask = wk.tile([E, 1], F32, tag="zmask")
                nc.vector.tensor_scalar(out=zmask[:], in0=uT[:, :1],
                                        scalar1=0.0, scalar2=None,
                                        op0=ALU.mult)
                uview = uall_t.ap().rearrange("e c b -> e (c b)")
                zma = zmask[:]
                zb = bass.AP(zma.tensor, zma.offset, [zma.ap[0], (0, c.B)])
                i_uz = nc.vector.tensor_tensor(out=uz[:], in0=uview, in1=zb,
                                               op=ALU.add)
                duf = wk.tile([E, c.B], F32, tag="duf")
                i_duf = nc.vector.tensor_tensor(out=duf[:], in0=uview,
                                                in1=zb, op=ALU.add)
                nc.sync.dma_start(out=t_du.ap(), in_=duf[:])
                rdma_fixups += [(i_uz, rsemU), (i_duf, rsemU)]
            else:
                ub_in = dram.tile([E, Bc], F32)
                ub_out = dram.tile([c.ncore * E, Bc], F32)
                nc.sync.dma_start(out=ub_in[:], in_=uT[:])
                nc.gpsimd.collective_compute(
                    "AllGather", ALU.bypass,
                    replica_groups=[list(range(c.ncore))],
                    ins=[ub_in.opt()], outs=[ub_out.opt()],
                )
                uTf = wk.tile([E, c.ncore, Bc], F32, tag="uTf")
                src = bass.AP(ub_out[:].tensor, ub_out[:].offset,
                              [(Bc, E), (E * Bc, c.ncore), (1, Bc)])
                nc.sync.dma_start(out=uTf[:], in_=src)
                nc.sync.dma_start(out=t_du.ap(),
                                  in_=uTf[:].rearrange("e c b -> e (c b)"))
                nc.vector.tensor_copy(uz[:],
                                      uTf[:].rearrange("e c b -> e (c b)"))

            # ---- logits + log_softmax (chunk pairs on 128 partitions) -----
            npar = VSH // 1024
            rem = VSH - npar * 1024
            assert rem <= 512, (VSH, npar, rem)
            nzc2 = npar + (1 if rem else 0)
            zW = 512 * npar + rem
            zbuf = big.tile([128, zW], F32)
            sums2 = wk.tile([128, max(nzc2, 1)], F32, tag="sums2")
            for i in range(npar):
                zps = psZ.tile([128, 512], F32, space="PSUM", tag="zps")
                nc.tensor.matmul(out=zps[:c.B, :], lhsT=uz[:],
                                 rhs=a3t[:, 1024 * i:1024 * i + 512],
                                 start=True, stop=True)
                nc.tensor.matmul(out=zps[c.B:128, :], lhsT=uz[:],
                                 rhs=a3t[:, 1024 * i + 512:1024 * (i + 1)],
                                 start=True, stop=True)
                esc = hp.tile([128, 512], DT, tag="esc")
                nc.scalar.activation(out=esc[:], in_=zps[:], func=AF.Exp,
                                     accum_out=sums2[:, i:i + 1])
                nc.vector.tensor_copy(zbuf[:, 512 * i:512 * (i + 1)], zps[:])
            if rem:
                zps = psZ.tile([128, 512], F32, space="PSUM", tag="zps")
                nc.tensor.matmul(out=zps[:c.B, :rem], lhsT=uz[:],
                                 rhs=a3t[:, npar * 1024:VSH],
                                 start=True, stop=True)
                esc = hp.tile([128, 512], DT, tag="esc")
                nc.scalar.activation(out=esc[:c.B, :rem],
                                     in_=zps[:c.B, :rem], func=AF.Exp,
                                     accum_out=sums2[:c.B, npar:npar + 1])
                nc.vector.tensor_copy(zbuf[:c.B, 512 * npar:zW],
                                      zps[:c.B, :rem])

            slcio = wk.tile([128, 1], F32, tag="slcio")
            nc.vector.tensor_reduce(out=slcio[:c.B, :],
                                    in_=sums2[:c.B, :nzc2], axis=AX.X,
                                    op=ALU.add)
            if npar > 0:
                nc.vector.tensor_reduce(out=slcio[c.B:128, :],
                                        in_=sums2[c.B:128, :npar], axis=AX.X,
                                        op=ALU.add)
            else:
                nc.vector.memset(slcio[c.B:128, :], 0.0)
            if c.rdma:
                for r in range(c.ncore):
                    with tc.If(rank == r):
                        nc.gpsimd.remote_dma_broadcast(
                            out_ap=sall_t.ap()[:, r:r + 1],
                            in_ap=slcio[:],
                            remote_sem=rsemS,
                            local_sem=lsemS,
                            rdests=[(0, k) for k in range(c.ncore)],
                        )
                        nc.gpsimd.trigger_dma(count=1)
                smask = wk.tile([128, 1], F32, tag="smask")
                nc.vector.tensor_scalar(out=smask[:], in0=slcio[:],
                                        scalar1=0.0, scalar2=None,
                                        op0=ALU.mult)
                sma = smask[:]
                sbb = bass.AP(sma.tensor, sma.offset,
                              [sma.ap[0], (0, c.ncore)])
                stmp = wk.tile([128, c.ncore], F32, tag="stmp")
                i_sm = nc.vector.tensor_tensor(out=stmp[:], in0=sall_t.ap(),
                                               in1=sbb, op=ALU.add)
                rdma_fixups.append((i_sm, rsemS))
                red = wk.tile([128, 1], F32, tag="red")
                nc.vector.tensor_reduce(out=red[:], in_=stmp[:],
                                        axis=AX.X, op=ALU.add)
                # swap partition halves so every lane has top+bottom sums
                redsw = wk.tile([128, 1], F32, tag="redsw")
                nc.sync.dma_start(out=redsw[:c.B, :], in_=red[c.B:128, :])
                nc.sync.dma_start(out=redsw[c.B:128, :], in_=red[:c.B, :])
                stF = wk.tile([128, 1], F32, tag="stF")
                nc.vector.tensor_tensor(out=stF[:], in0=red[:], in1=redsw[:],
                                        op=ALU.add)
            else:
                sb_in = dram.tile([128, 1], F32)
                sb_out = dram.tile([128, 1], F32)
                nc.sync.dma_start(out=sb_in[:], in_=slcio[:])
                nc.gpsimd.collective_compute(
                    "AllReduce", ALU.add,
                    replica_groups=[list(range(c.ncore))],
                    ins=[sb_in.opt()], outs=[sb_out.opt()],
                )
                stA = wk.tile([128, 1], F32, tag="stA")
                stB = wk.tile([128, 1], F32, tag="stB")
                so = sb_out[:]
                nc.sync.dma_start(out=stA[:c.B, :], in_=so[:c.B])
                nc.sync.dma_start(out=stA[c.B:128, :], in_=so[:c.B])
                nc.sync.dma_start(out=stB[:c.B, :], in_=so[c.B:128])
                nc.sync.dma_start(out=stB[c.B:128, :], in_=so[c.B:128])
                stF = wk.tile([128, 1], F32, tag="stF")
                nc.vector.tensor_tensor(out=stF[:], in0=stA[:], in1=stB[:],
                                        op=ALU.add)
            lseB = wk.tile([128, 1], F32, tag="lseB")
            nc.scalar.activation(out=lseB[:], in_=stF[:], func=AF.Ln)

            # subtract lse and stream out (bf16), sliced so DVE/DMA overlap
            obuf = big.tile([128, zW], ODT)
            PSL = 4
            i0 = 0
            while i0 < npar:
                i1 = min(i0 + PSL, npar)
                nc.vector.tensor_scalar(
                    out=obuf[:, 512 * i0:512 * i1],
                    in0=zbuf[:, 512 * i0:512 * i1],
                    scalar1=lseB[:], scalar2=None, op0=ALU.subtract)
                dst_e = bass.AP(t_o.ap().tensor, 1024 * i0,
                                [(VSH, c.B), (1024, i1 - i0), (1, 512)])
                nc.sync.dma_start(
                    out=dst_e,
                    in_=obuf[:c.B, 512 * i0:512 * i1].rearrange(
                        "b (i j) -> b i j", j=512))
                dst_o = bass.AP(t_o.ap().tensor, 1024 * i0 + 512,
                                [(VSH, c.B), (1024, i1 - i0), (1, 512)])
                nc.sync.dma_start(
                    out=dst_o,
                    in_=obuf[c.B:128, 512 * i0:512 * i1].rearrange(
                        "b (i j) -> b i j", j=512))
                i0 = i1
            if rem:
                nc.vector.tensor_scalar(
                    out=obuf[:c.B, 512 * npar:zW],
                    in0=zbuf[:c.B, 512 * npar:zW],
                    scalar1=lseB[:c.B, :], scalar2=None, op0=ALU.subtract)
                dst_r = bass.AP(t_o.ap().tensor, 1024 * npar,
                                [(VSH, c.B), (1, rem)])
                nc.sync.dma_start(out=dst_r, in_=obuf[:c.B, 512 * npar:zW])

    if c.rdma:
        # Attach the remote-arrival gates AFTER Tile scheduling: the
        # single-core scheduling sim cannot satisfy waits fed by peers'
        # remote DMAs (it would report a deadlock), but the hardware can.
        for inst, sem in rdma_fixups:
            target = inst.ins
            placed = False
            for blk in nc.main_func.blocks:
                for idx, i2 in enumerate(blk.instructions):
                    if i2 is target:
                        w = mybir.SyncWait(
                            sync_type="semaphore", id=sem.num,
                            wait_mode="sem-ge-imm", wait_value=16,
                            ant_name=sem.name)
                        ev = mybir.InstEventSemaphore(
                            name=nc.get_next_instruction_name(),
                            ins=[], outs=[])
                        ev.engine = target.engine
                        ev.sync_info = mybir.SyncInfo(on_wait=[w],
                                                      on_update=[])
                        nc.register_instruction(ev)
                        blk.instructions.insert(idx, ev)
                        placed = True
                        break
                if placed:
                    break
            assert placed, "rdma wait target instruction not found"

    nc.compile()
    return nc


def host_prep(cfg, x, q, A, TA, TC):
    c = cfg
    E, J, S = c.E, c.J, c.S
    x = np.asarray(x).astype(np.int64)
    q = np.asarray(q).astype(np.int64)
    A = np.asarray(A, dtype=np.float32)
    TA = np.asarray(TA, dtype=np.float32)
    TC = np.asarray(TC, dtype=np.float32)

    tabI = np.ascontiguousarray(A.transpose(1, 0, 2).reshape(c.V, 4 * E))
    wscale = 1.0
    if c.g_fp8:
        # prescale x16 lifts N(0,0.1) values out of e4m3 denormal range;
        # the /16 on the one-hot PE weights cancels it exactly in the matmul
        tabI = (tabI * 16.0).astype(c.gnp)
        wscale = 1.0 / 16.0
    else:
        tabI = tabI.astype(c.npdt)
    a3tF = np.ascontiguousarray(A[3].T)  # [E, V] f32
    if c.z_fp8:
        a3tF = a3tF * 16.0

    j = np.arange(1, J + 1, dtype=np.float32)
    av = 1.0 - j / J
    bv = 2.0 * j / J - 1.0
    sp = np.arange(128) // J
    jj = np.arange(128) % J
    wab = np.zeros((128, 2 * c.SPT), np.float32)
    wabc = np.zeros((128, 3 * c.SPT), np.float32)
    wc = np.zeros((128, c.SPT), np.float32)
    for p in range(128):
        wab[p, 2 * sp[p] + 0] = av[jj[p]]
        wab[p, 2 * sp[p] + 1] = bv[jj[p]]
        wabc[p, 3 * sp[p] + 0] = av[jj[p]]
        wabc[p, 3 * sp[p] + 1] = bv[jj[p]]
        wabc[p, 3 * sp[p] + 2] = 1.0
        wc[p, sp[p]] = 1.0
    wq = np.zeros((128, c.Bc), np.float32)
    for p in range(128):
        wq[p, p // c.QW] = 1.0

    tat = np.ascontiguousarray(TA[0, :S, :].T)   # [E, S]
    tct = np.ascontiguousarray(TC[0, :S, :].T)
    tatn = np.tile(tat, (1, c.Bc))               # [E, NS] batch-major
    tctn = np.tile(tct, (1, c.Bc))
    kp = ((np.arange(E, dtype=np.float32) + 1.0) / E).reshape(E, 1)
    mask2 = np.full((128, c.nblk * c.Bc), NEG, np.float32)
    for k in range(c.nblk):
        for p in range(128):
            s = k * 128 + p
            if s < c.NS:
                mask2[p, k * c.Bc + s // S] = 0.0
    onesc = np.ones((128, 1), np.float32)
    ones1 = np.ones((1, 128), np.float32)

    common = {
        "wab": (wab * wscale).astype(c.gnp),
        "wabc": (wabc * wscale).astype(c.gnp),
        "wc": (wc * wscale).astype(c.gnp),
        "wq": (wq * wscale).astype(c.gnp),
        "tatn": np.ascontiguousarray(tatn),
        "tctn": np.ascontiguousarray(tctn),
        "kp": kp, "mask2": mask2,
        "onesc": onesc.astype(c.npdt), "ones1": ones1,
    }

    nch = len(c.gsizes)
    in_maps = []
    for cc in range(c.ncore):
        xc = x[cc * c.Bc:(cc + 1) * c.Bc].reshape(-1)
        qc = q[cc * c.Bc:(cc + 1) * c.Bc].reshape(-1)
        xq = np.concatenate([xc, qc])
        uniq, rel = np.unique(xq, return_inverse=True)
        assert len(uniq) <= c.ucap, (len(uniq), c.ucap)
        tabc = np.zeros((c.ucap, 4 * E), c.gnp)
        tabc[:len(uniq)] = tabI[uniq]
        rel = rel.astype(np.int16)
        idx = np.zeros((nch, 128, c.gchunk // 16), np.int16)
        off = 0
        for g, gs in enumerate(c.gsizes):
            v = rel[off:off + gs]
            off += gs
            wrapped = v.reshape(-1, 16).T
            idx[g, :, : gs // 16] = np.tile(wrapped, (8, 1))
        a3c = np.ascontiguousarray(
            a3tF[:, cc * c.VSH:(cc + 1) * c.VSH]).astype(c.znp)
        m = dict(common)
        m.update({"tabc": tabc, "idx": idx, "a3t": a3c})
        in_maps.append(m)
    return in_maps


_CACHE = {}


def _get_module(cfg):
    k = cfg.key()
    if k not in _CACHE:
        _CACHE[k] = build_module(cfg)
    return _CACHE[k]


def run(cfg, inputs, trace=False):
    nc = _get_module(cfg)
    in_maps = host_prep(cfg, inputs["x"], inputs["q"], inputs["A"],
                        inputs["TA"], inputs["TC"])
    res = bass_utils.run_bass_kernel_spmd(
        nc, in_maps, core_ids=list(range(cfg.ncore)), trace=trace)
    out = np.concatenate(
        [np.asarray(res.results[cc]["o"]).astype(np.float32)
         for cc in range(cfg.ncore)], axis=1)
    return out, res


def kernel(**inputs) -> np.ndarray:
    cfg = Cfg()
    out, _ = run(cfg, inputs, trace=False)
    return out


# revision 49
# speedup vs baseline: 1.2550x; 1.0650x over previous
"""MemNN (embedding_lookup) Trainium2 Bass kernel.

Strategy (8 NeuronCores, one NEFF, SPMD):
  - Data-parallel hops: batch dim sharded 8 ways (8 batches/core).
  - Host packs the 4 embedding tables interleaved per vocab row
    ([A0|A1|A2|A3][v], bf16) and, per core, compacts it to the core's
    unique vocab rows so indices fit dma_gather's int16 (~22.6K < 32767).
  - dma_gather streams all (story + query) embedding rows across the
    4 SWDGE queues (round-robin) so descriptor generation runs on all
    4 Q7 core-pairs concurrently; PE matmuls with fixed block weights
    reduce each 128-row tile into per-sentence partial sums (PSUM).
    Position encoding is rank-2 separable:
        pe[j,d] = a(j) + b(j) * k'(d),  a=1-j/J, b=2j/J-1, k'=d/D
    so m = S_a + k' * S_b needs only two weighted sums per sentence.
    Temporal encodings are folded into the per-chunk combines.
  - c sums are DMA-transposed (in-stream, under the gather) into
    sentence-major cN[s,E] so the hops need no transposes at all:
    scores come out of PE directly as [sentence, batch], softmax runs
    unnormalized (exp + ones-matmul column sums + reciprocal), and the
    u update contracts over sentence partitions.
  - AllGather u across cores; vocab-sharded logits z = u @ A3^T with
    chunk pairs packed into all 128 PSUM partitions; log_softmax via
    AllReduce of exp-sums; final subtract/writeout pipelined.
"""

import numpy as np
import ml_dtypes

import concourse.bass as bass
import concourse.mybir as mybir
import concourse.tile as tile
from concourse import bacc
import concourse.bass_utils as bass_utils

F32 = mybir.dt.float32
AF = mybir.ActivationFunctionType
ALU = mybir.AluOpType
AX = mybir.AxisListType

NEG = -1e30


class Cfg:
    def __init__(self, ncore=8, B=64, S=50, J=64, QW=16, V=100000, E=128,
                 ucap=24576, gchunk=1024, use_bf16=True, z_f32=False,
                 g_fp8=False, z_fp8=False, rdma=False):
        self.ncore, self.B, self.S, self.J, self.QW = ncore, B, S, J, QW
        self.V, self.E, self.ucap, self.gchunk = V, E, ucap, gchunk
        self.use_bf16, self.z_f32 = use_bf16, z_f32
        self.g_fp8, self.z_fp8 = g_fp8, z_fp8
        self.rdma = rdma
        self.Bc = B // ncore
        self.NS = self.Bc * S              # sentences per core
        self.NX = self.NS * J              # story rows per core
        self.NQ = self.Bc * QW             # query rows per core
        assert self.NQ == 128 and self.NX % 128 == 0
        self.NPOS = self.NX + self.NQ
        self.xtiles = self.NX // 128
        self.SPT = 128 // J                # sentences per 128-row tile
        assert 128 % J == 0
        self.VSH = V // ncore
        sizes = []
        rem = self.NPOS
        while rem > 0:
            s = min(gchunk, rem)
            sizes.append(s)
            rem -= s
        assert all(s % 128 == 0 for s in sizes)
        self.gsizes = sizes
        self.nblk = (self.NS + 127) // 128  # 128-sentence blocks
        self.NSP = self.nblk * 128
        self.DT = mybir.dt.bfloat16 if use_bf16 else mybir.dt.float32
        self.npdt = ml_dtypes.bfloat16 if use_bf16 else np.float32
        # gather-table dtype (tables prescaled x16, weights /16 so the
        # PE reduction cancels the scale exactly)
        self.gdt = mybir.dt.float8e4 if g_fp8 else self.DT
        self.gnp = ml_dtypes.float8_e4m3 if g_fp8 else self.npdt
        assert not (z_f32 and z_fp8)
        self.zdt = F32 if z_f32 else (mybir.dt.float8e4 if z_fp8 else self.DT)
        self.znp = (np.float32 if z_f32 else
                    (ml_dtypes.float8_e4m3 if z_fp8 else self.npdt))

    def key(self):
        return (self.ncore, self.B, self.S, self.J, self.QW, self.V, self.E,
                self.ucap, self.gchunk, self.use_bf16, self.z_f32,
                self.g_fp8, self.z_fp8, self.rdma)


def build_module(cfg):
    c = cfg
    E, NS, Bc, VSH, S = c.E, c.NS, c.Bc, c.VSH, c.S
    NSP, nblk = c.NSP, c.nblk
    DT = c.DT
    nc = bacc.Bacc("TRN2", target_bir_lowering=False, debug=False,
                   num_devices=c.ncore, num_swdge_queues=4)

    GDT = c.gdt
    t_tab = nc.dram_tensor("tabc", [c.ucap, 4 * E], GDT, kind="ExternalInput")
    nch = len(c.gsizes)
    t_idx = nc.dram_tensor("idx", [nch, 128, c.gchunk // 16], mybir.dt.int16,
                           kind="ExternalInput")
    t_a3t = nc.dram_tensor("a3t", [E, VSH], c.zdt, kind="ExternalInput")
    t_wab = nc.dram_tensor("wab", [128, 2 * c.SPT], GDT, kind="ExternalInput")
    t_wabc = nc.dram_tensor("wabc", [128, 3 * c.SPT], GDT,
                            kind="ExternalInput")
    t_wc = nc.dram_tensor("wc", [128, c.SPT], GDT, kind="ExternalInput")
    t_wq = nc.dram_tensor("wq", [128, Bc], GDT, kind="ExternalInput")
    t_tatn = nc.dram_tensor("tatn", [E, NS], F32, kind="ExternalInput")
    t_tctn = nc.dram_tensor("tctn", [E, NS], F32, kind="ExternalInput")
    t_kp = nc.dram_tensor("kp", [E, 1], F32, kind="ExternalInput")
    t_mask2 = nc.dram_tensor("mask2", [128, nblk * Bc], F32,
                             kind="ExternalInput")
    t_ones = nc.dram_tensor("onesc", [128, 1], DT, kind="ExternalInput")
    t_ones1 = nc.dram_tensor("ones1", [1, 128], F32, kind="ExternalInput")

    ODT = DT if c.use_bf16 else F32
    t_o = nc.dram_tensor("o", [c.B, VSH], ODT, kind="ExternalOutput")
    t_du = nc.dram_tensor("du", [E, c.B], F32, kind="ExternalOutput")

    if c.rdma:
        # Raw (non-pool) SBUF receive buffers: written by REMOTE cores'
        # broadcasts, so Tile must not dep-track them — the explicit
        # wait_ge on the remote sem is the only correct gate.
        uall_t = nc.alloc_sbuf_tensor("uall", [E, c.ncore, Bc], F32)
        sall_t = nc.alloc_sbuf_tensor("sall", [128, c.ncore], F32)
        rsemU = nc.alloc_semaphore("rsemU")
        lsemU = nc.alloc_semaphore("lsemU")
        rsemS = nc.alloc_semaphore("rsemS")
        lsemS = nc.alloc_semaphore("lsemS")

    with tile.TileContext(nc) as tc:
        with tc.tile_pool(name="const", bufs=1) as cpool, \
             tc.tile_pool(name="gp", bufs=8) as gpool, \
             tc.tile_pool(name="wk", bufs=1) as wk, \
             tc.tile_pool(name="hp", bufs=2) as hp, \
             tc.tile_pool(name="big", bufs=1) as big, \
             tc.tile_pool(name="psG", bufs=3, space="PSUM") as psG, \
             tc.tile_pool(name="psH", bufs=1, space="PSUM") as psH, \
             tc.tile_pool(name="psZ", bufs=3, space="PSUM") as psZ, \
             tc.tile_pool(name="dram", bufs=1, space="DRAM") as dram:

            # ---- constant loads -------------------------------------------
            # The gather indices go FIRST on the sync queue (they alone gate
            # the gather stream); the small consts follow and still arrive
            # before the first chunk's PE reductions need them; the big a3t
            # load goes on the scalar HWDGE queue out of the way.
            idxs = cpool.tile([128, nch, c.gchunk // 16], mybir.dt.int16)
            nc.sync.dma_start(out=idxs[:],
                              in_=t_idx.ap().rearrange("g p w -> p g w"))
            a3t = big.tile([E, VSH], c.zdt)
            nc.scalar.dma_start(out=a3t[:], in_=t_a3t.ap())
            wab = cpool.tile([128, 2 * c.SPT], GDT)
            nc.sync.dma_start(out=wab[:], in_=t_wab.ap())
            wabc = cpool.tile([128, 3 * c.SPT], GDT)
            nc.sync.dma_start(out=wabc[:], in_=t_wabc.ap())
            wc_t = cpool.tile([128, c.SPT], GDT)
            nc.sync.dma_start(out=wc_t[:], in_=t_wc.ap())
            wq = cpool.tile([128, Bc], GDT)
            nc.sync.dma_start(out=wq[:], in_=t_wq.ap())
            tatn = cpool.tile([E, NS], F32)
            nc.sync.dma_start(out=tatn[:], in_=t_tatn.ap())
            tctn = cpool.tile([E, NS], F32)
            nc.sync.dma_start(out=tctn[:], in_=t_tctn.ap())
            kp = cpool.tile([E, 1], F32)
            nc.sync.dma_start(out=kp[:], in_=t_kp.ap())
            mask2 = cpool.tile([128, nblk * Bc], F32)
            nc.sync.dma_start(out=mask2[:], in_=t_mask2.ap())
            onesc = cpool.tile([128, 1], DT)
            nc.sync.dma_start(out=onesc[:], in_=t_ones.ap())
            ones1 = cpool.tile([1, 128], F32)
            nc.sync.dma_start(out=ones1[:], in_=t_ones1.ap())

            # warm the Exp activation table during the gather phase
            dume = wk.tile([1, 1], F32, tag="dume")
            nc.scalar.activation(out=dume[:], in_=kp[:1, :1], func=AF.Exp)

            # ---- persistent work tiles ------------------------------------
            mT = [wk.tile([E, NSP], F32, tag=f"mT{h}", name=f"mT{h}")
                  for h in range(3)]
            cT = [wk.tile([E, NSP], DT, tag=f"cT{h}", name=f"cT{h}")
                  for h in range(3)]
            cN = [wk.tile([128, nblk, E], DT, tag=f"cN{h}", name=f"cN{h}")
                  for h in range(3)]
            uT = wk.tile([E, Bc], F32, tag="uT")
            if NSP > NS:
                for h in range(3):
                    nc.vector.memset(mT[h][:, NS:NSP], 0.0)
                    nc.vector.memset(cT[h][:, NS:NSP], 0.0)

            # ---- gather + per-chunk reductions + combines -----------------
            tile_idx = 0
            done_blk = 0
            for g, gs in enumerate(c.gsizes):
                slots = gs // 128
                nxt = min(slots, c.xtiles - tile_idx)   # x-tiles this chunk
                has_q = (tile_idx + slots) > c.xtiles
                spg = nxt * c.SPT
                gs0 = tile_idx * c.SPT
                L0, L1, L2 = 0, 2 * spg, 5 * spg
                L3, Lq = 8 * spg, 9 * spg

                gt = gpool.tile([128, c.gchunk // 128, 4 * E], GDT, tag="g")
                nc.gpsimd.dma_gather(
                    out_ap=gt[:, :slots, :],
                    in_ap=t_tab.ap(),
                    idxs_ap=idxs[:, g, : gs // 16],
                    num_idxs=gs,
                    num_idxs_reg=gs,
                    elem_size=4 * E,
                    queue_num=g % 4,
                )
                Pg = psG.tile([128, 512], F32, space="PSUM", tag="Pg")
                for sl in range(slots):
                    t = tile_idx
                    tile_idx += 1
                    if t < c.xtiles:
                        ls0 = (t * c.SPT) - gs0
                        G0 = gt[:, sl, 0 * E:1 * E]
                        G1 = gt[:, sl, 1 * E:2 * E]
                        G2 = gt[:, sl, 2 * E:3 * E]
                        G3 = gt[:, sl, 3 * E:4 * E]
                        nc.tensor.matmul(
                            out=Pg[:, L0 + 2 * ls0: L0 + 2 * ls0 + 2 * c.SPT],
                            lhsT=G0, rhs=wab[:], start=True, stop=True)
                        nc.tensor.matmul(
                            out=Pg[:, L1 + 3 * ls0: L1 + 3 * ls0 + 3 * c.SPT],
                            lhsT=G1, rhs=wabc[:], start=True, stop=True)
                        nc.tensor.matmul(
                            out=Pg[:, L2 + 3 * ls0: L2 + 3 * ls0 + 3 * c.SPT],
                            lhsT=G2, rhs=wabc[:], start=True, stop=True)
                        nc.tensor.matmul(
                            out=Pg[:, L3 + ls0: L3 + ls0 + c.SPT],
                            lhsT=G3, rhs=wc_t[:], start=True, stop=True)
                    else:
                        nc.tensor.matmul(
                            out=Pg[:, Lq: Lq + Bc],
                            lhsT=gt[:, sl, 0 * E:1 * E], rhs=wq[:],
                            start=True, stop=True)

                # per-chunk combines (psum -> sbuf slices, encodings folded)
                pap = Pg[:]
                pdim = pap.ap[0]

                def pv(base, gw, off, n=spg):
                    return bass.AP(pap.tensor, pap.offset + base + off,
                                   [pdim, (gw, n)])

                if spg > 0:
                    for h, (base, gw) in enumerate(
                            [(L0, 2), (L1, 3), (L2, 3)]):
                        msl = mT[h][:, gs0:gs0 + spg]
                        nc.vector.tensor_scalar(
                            out=msl, in0=pv(base, gw, 1), scalar1=kp[:],
                            scalar2=None, op0=ALU.mult)
                        nc.vector.tensor_tensor(
                            out=msl, in0=msl, in1=pv(base, gw, 0),
                            op=ALU.add)
                        nc.vector.tensor_tensor(
                            out=msl, in0=msl, in1=tatn[:, gs0:gs0 + spg],
                            op=ALU.add)
                    for h, (base, gw, off) in enumerate(
                            [(L1, 3, 2), (L2, 3, 2), (L3, 1, 0)]):
                        csl = cT[h][:, gs0:gs0 + spg]
                        nc.vector.tensor_tensor(
                            out=csl, in0=pv(base, gw, off),
                            in1=tctn[:, gs0:gs0 + spg], op=ALU.add)
                if has_q:
                    nc.vector.tensor_copy(uT[:], Pg[:, Lq: Lq + Bc])

                # DMA-transpose completed 128-sentence blocks of c into
                # sentence-major cN while the gather stream continues.
                end = gs0 + spg
                while done_blk < nblk and end >= min((done_blk + 1) * 128, NS):
                    b = done_blk
                    for h in range(3):
                        # scalar HWDGE queue: its inline waits must not block
                        # the sync queue that feeds everything else
                        nc.scalar.dma_start(
                            out=cN[h][:, b, :],
                            in_=cT[h][:, 128 * b:128 * (b + 1)],
                            transpose=True)
                    done_blk += 1

            # ---- hops (transpose-free) ------------------------------------
            for h in range(3):
                S_ps = psH.tile([128, (nblk + 1) * Bc], F32, space="PSUM",
                                tag="sc")
                for k in range(nblk):
                    nc.tensor.matmul(out=S_ps[:, k * Bc:(k + 1) * Bc],
                                     lhsT=mT[h][:, 128 * k:128 * (k + 1)],
                                     rhs=uT[:], start=True, stop=True)
                ex = hp.tile([128, nblk * Bc], F32, tag="ex")
                nc.vector.tensor_tensor(out=ex[:], in0=S_ps[:, :nblk * Bc],
                                        in1=mask2[:], op=ALU.add)
                P = hp.tile([128, nblk, Bc], DT, tag="P")
                nc.scalar.activation(out=P[:].rearrange("p k b -> p (k b)"),
                                     in_=ex[:], func=AF.Exp)
                U_ps = psH.tile([E, 2 * Bc], F32, space="PSUM", tag="up")
                for k in range(nblk):
                    nc.tensor.matmul(out=U_ps[:, :Bc], lhsT=cN[h][:, k, :],
                                     rhs=P[:, k, :], start=(k == 0),
                                     stop=(k == nblk - 1))
                for k in range(nblk):
                    nc.tensor.matmul(
                        out=S_ps[:1, nblk * Bc:(nblk + 1) * Bc],
                        lhsT=onesc[:], rhs=P[:, k, :], start=(k == 0),
                        stop=(k == nblk - 1))
                rz1 = hp.tile([1, Bc], F32, tag="rz1")
                nc.vector.reciprocal(
                    out=rz1[:], in_=S_ps[:1, nblk * Bc:(nblk + 1) * Bc])
                nc.tensor.matmul(out=U_ps[:, Bc:2 * Bc], lhsT=ones1[:],
                                 rhs=rz1[:], start=True, stop=True)
                rzc = hp.tile([E, Bc], F32, tag="rzc")
                nc.vector.tensor_copy(rzc[:], U_ps[:, Bc:2 * Bc])
                un = wk.tile([E, Bc], F32, tag=f"uT{h + 1}")
                nc.vector.tensor_tensor(out=un[:], in0=U_ps[:, :Bc],
                                        in1=rzc[:], op=ALU.mult)
                nc.vector.tensor_tensor(out=un[:], in0=un[:], in1=uT[:],
                                        op=ALU.add)
                uT = un

            # ---- AllGather u ----------------------------------------------
            uz = wk.tile([E, c.B], c.zdt, tag="uz")
            rdma_fixups = []
            if c.rdma:
                # direct 1-hop allgather: each core broadcasts its u into
                # its own rank slot on every peer (incl. self); every
                # receiver's rsemU reaches 16 when all 8 frames landed
                rank = nc.gpsimd.partition_id()
                for r in range(c.ncore):
                    with tc.If(rank == r):
                        nc.gpsimd.remote_dma_broadcast(
                            out_ap=uall_t.ap()[:, r, :],
                            in_ap=uT[:],
                            remote_sem=rsemU,
                            local_sem=lsemU,
                            rdests=[(0, k) for k in range(c.ncore)],
                        )
                        nc.gpsimd.trigger_dma(count=1)
                # zero token written after the hops gives the consumers a
                # tracked dep (keeps scheduler order); the remote-arrival
                # gate (rsemU >= 16) is appended post-schedule.
                zmask = wk.tile([E, 1], F32, tag="zmask")
                nc.vector.tensor_scalar(out=zmask[:], in0=uT[:, :1],
                                        scalar1=0.0, scalar2=None,
                                        op0=ALU.mult)
                uview = uall_t.ap().rearrange("e c b -> e (c b)")
                zma = zmask[:]
                zb = bass.AP(zma.tensor, zma.offset, [zma.ap[0], (0, c.B)])
                i_uz = nc.vector.tensor_tensor(out=uz[:], in0=uview, in1=zb,
                                               op=ALU.add)
                duf = wk.tile([E, c.B], F32, tag="duf")
                i_duf = nc.vector.tensor_tensor(out=duf[:], in0=uview,
                                                in1=zb, op=ALU.add)
                nc.sync.dma_start(out=t_du.ap(), in_=duf[:])
                rdma_fixups += [(i_uz, rsemU), (i_duf, rsemU)]
            else:
                ub_in = dram.tile([E, Bc], F32)
                ub_out = dram.tile([c.ncore * E, Bc], F32)
                nc.sync.dma_start(out=ub_in[:], in_=uT[:])
                nc.gpsimd.collective_compute(
                    "AllGather", ALU.bypass,
                    replica_groups=[list(range(c.ncore))],
                    ins=[ub_in.opt()], outs=[ub_out.opt()],
                )
                uTf = wk.tile([E, c.ncore, Bc], F32, tag="uTf")
                src = bass.AP(ub_out[:].tensor, ub_out[:].offset,
                              [(Bc, E), (E * Bc, c.ncore), (1, Bc)])
                nc.sync.dma_start(out=uTf[:], in_=src)
                nc.sync.dma_start(out=t_du.ap(),
                                  in_=uTf[:].rearrange("e c b -> e (c b)"))
                nc.vector.tensor_copy(uz[:],
                                      uTf[:].rearrange("e c b -> e (c b)"))

            # ---- logits + log_softmax (chunk pairs on 128 partitions) -----
            npar = VSH // 1024
            rem = VSH - npar * 1024
            assert rem <= 512, (VSH, npar, rem)
            nzc2 = npar + (1 if rem else 0)
            zW = 512 * npar + rem
            zbuf = big.tile([128, zW], F32)
            sums2 = wk.tile([128, max(nzc2, 1)], F32, tag="sums2")
            for i in range(npar):
                zps = psZ.tile([128, 512], F32, space="PSUM", tag="zps")
                nc.tensor.matmul(out=zps[:c.B, :], lhsT=uz[:],
                                 rhs=a3t[:, 1024 * i:1024 * i + 512],
                                 start=True, stop=True)
                nc.tensor.matmul(out=zps[c.B:128, :], lhsT=uz[:],
                                 rhs=a3t[:, 1024 * i + 512:1024 * (i + 1)],
                                 start=True, stop=True)
                esc = hp.tile([128, 512], DT, tag="esc")
                nc.scalar.activation(out=esc[:], in_=zps[:], func=AF.Exp,
                                     accum_out=sums2[:, i:i + 1])
                nc.vector.tensor_copy(zbuf[:, 512 * i:512 * (i + 1)], zps[:])
            if rem:
                zps = psZ.tile([128, 512], F32, space="PSUM", tag="zps")
                nc.tensor.matmul(out=zps[:c.B, :rem], lhsT=uz[:],
                                 rhs=a3t[:, npar * 1024:VSH],
                                 start=True, stop=True)
                esc = hp.tile([128, 512], DT, tag="esc")
                nc.scalar.activation(out=esc[:c.B, :rem],
                                     in_=zps[:c.B, :rem], func=AF.Exp,
                                     accum_out=sums2[:c.B, npar:npar + 1])
                nc.vector.tensor_copy(zbuf[:c.B, 512 * npar:zW],
                                      zps[:c.B, :rem])

            slcio = wk.tile([128, 1], F32, tag="slcio")
            nc.vector.tensor_reduce(out=slcio[:c.B, :],
                                    in_=sums2[:c.B, :nzc2], axis=AX.X,
                                    op=ALU.add)
            if npar > 0:
                nc.vector.tensor_reduce(out=slcio[c.B:128, :],
                                        in_=sums2[c.B:128, :npar], axis=AX.X,
                                        op=ALU.add)
            else:
                nc.vector.memset(slcio[c.B:128, :], 0.0)
            if c.rdma:
                for r in range(c.ncore):
                    with tc.If(rank == r):
                        nc.gpsimd.remote_dma_broadcast(
                            out_ap=sall_t.ap()[:, r:r + 1],
                            in_ap=slcio[:],
                            remote_sem=rsemS,
                            local_sem=lsemS,
                            rdests=[(0, k) for k in range(c.ncore)],
                        )
                        nc.gpsimd.trigger_dma(count=1)
                smask = wk.tile([128, 1], F32, tag="smask")
                nc.vector.tensor_scalar(out=smask[:], in0=slcio[:],
                                        scalar1=0.0, scalar2=None,
                                        op0=ALU.mult)
                sma = smask[:]
                sbb = bass.AP(sma.tensor, sma.offset,
                              [sma.ap[0], (0, c.ncore)])
                stmp = wk.tile([128, c.ncore], F32, tag="stmp")
                i_sm = nc.vector.tensor_tensor(out=stmp[:], in0=sall_t.ap(),
                                               in1=sbb, op=ALU.add)
                rdma_fixups.append((i_sm, rsemS))
                red = wk.tile([128, 1], F32, tag="red")
                nc.vector.tensor_reduce(out=red[:], in_=stmp[:],
                                        axis=AX.X, op=ALU.add)
                # swap partition halves so every lane has top+bottom sums
                redsw = wk.tile([128, 1], F32, tag="redsw")
                nc.sync.dma_start(out=redsw[:c.B, :], in_=red[c.B:128, :])
                nc.sync.dma_start(out=redsw[c.B:128, :], in_=red[:c.B, :])
                stF = wk.tile([128, 1], F32, tag="stF")
                nc.vector.tensor_tensor(out=stF[:], in0=red[:], in1=redsw[:],
                                        op=ALU.add)
            else:
                sb_in = dram.tile([128, 1], F32)
                sb_out = dram.tile([128, 1], F32)
                nc.sync.dma_start(out=sb_in[:], in_=slcio[:])
                nc.gpsimd.collective_compute(
                    "AllReduce", ALU.add,
                    replica_groups=[list(range(c.ncore))],
                    ins=[sb_in.opt()], outs=[sb_out.opt()],
                )
                stA = wk.tile([128, 1], F32, tag="stA")
                stB = wk.tile([128, 1], F32, tag="stB")
                so = sb_out[:]
                nc.sync.dma_start(out=stA[:c.B, :], in_=so[:c.B])
                nc.sync.dma_start(out=stA[c.B:128, :], in_=so[:c.B])
                nc.sync.dma_start(out=stB[:c.B, :], in_=so[c.B:128])
                nc.sync.dma_start(out=stB[c.B:128, :], in_=so[c.B:128])
                stF = wk.tile([128, 1], F32, tag="stF")
                nc.vector.tensor_tensor(out=stF[:], in0=stA[:], in1=stB[:],
                                        op=ALU.add)
            lseB = wk.tile([128, 1], F32, tag="lseB")
            nc.scalar.activation(out=lseB[:], in_=stF[:], func=AF.Ln)

            # subtract lse and stream out (bf16), sliced so DVE/DMA overlap
            obuf = big.tile([128, zW], ODT)
            PSL = 4
            i0 = 0
            while i0 < npar:
                i1 = min(i0 + PSL, npar)
                nc.vector.tensor_scalar(
                    out=obuf[:, 512 * i0:512 * i1],
                    in0=zbuf[:, 512 * i0:512 * i1],
                    scalar1=lseB[:], scalar2=None, op0=ALU.subtract)
                dst_e = bass.AP(t_o.ap().tensor, 1024 * i0,
                                [(VSH, c.B), (1024, i1 - i0), (1, 512)])
                nc.sync.dma_start(
                    out=dst_e,
                    in_=obuf[:c.B, 512 * i0:512 * i1].rearrange(
                        "b (i j) -> b i j", j=512))
                dst_o = bass.AP(t_o.ap().tensor, 1024 * i0 + 512,
                                [(VSH, c.B), (1024, i1 - i0), (1, 512)])
                nc.sync.dma_start(
                    out=dst_o,
                    in_=obuf[c.B:128, 512 * i0:512 * i1].rearrange(
                        "b (i j) -> b i j", j=512))
                i0 = i1
            if rem:
                nc.vector.tensor_scalar(
                    out=obuf[:c.B, 512 * npar:zW],
                    in0=zbuf[:c.B, 512 * npar:zW],
                    scalar1=lseB[:c.B, :], scalar2=None, op0=ALU.subtract)
                dst_r = bass.AP(t_o.ap().tensor, 1024 * npar,
                                [(VSH, c.B), (1, rem)])
                nc.sync.dma_start(out=dst_r, in_=obuf[:c.B, 512 * npar:zW])

    if c.rdma:
        # Attach the remote-arrival gates AFTER Tile scheduling: the
        # single-core scheduling sim cannot satisfy waits fed by peers'
        # remote DMAs (it would report a deadlock), but the hardware can.
        for inst, sem in rdma_fixups:
            target = inst.ins
            placed = False
            for blk in nc.main_func.blocks:
                for idx, i2 in enumerate(blk.instructions):
                    if i2 is target:
                        w = mybir.SyncWait(
                            sync_type="semaphore", id=sem.num,
                            wait_mode="sem-ge-imm", wait_value=16,
                            ant_name=sem.name)
                        ev = mybir.InstEventSemaphore(
                            name=nc.get_next_instruction_name(),
                            ins=[], outs=[])
                        ev.engine = target.engine
                        ev.sync_info = mybir.SyncInfo(on_wait=[w],
                                                      on_update=[])
                        nc.register_instruction(ev)
                        blk.instructions.insert(idx, ev)
                        placed = True
                        break
                if placed:
                    break
            assert placed, "rdma wait target instruction not found"

    nc.compile()
    return nc


def host_prep(cfg, x, q, A, TA, TC):
    c = cfg
    E, J, S = c.E, c.J, c.S
    x = np.asarray(x).astype(np.int64)
    q = np.asarray(q).astype(np.int64)
    A = np.asarray(A, dtype=np.float32)
    TA = np.asarray(TA, dtype=np.float32)
    TC = np.asarray(TC, dtype=np.float32)

    tabI = np.ascontiguousarray(A.transpose(1, 0, 2).reshape(c.V, 4 * E))
    wscale = 1.0
    if c.g_fp8:
        # prescale x16 lifts N(0,0.1) values out of e4m3 denormal range;
        # the /16 on the one-hot PE weights cancels it exactly in the matmul
        tabI = (tabI * 16.0).astype(c.gnp)
        wscale = 1.0 / 16.0
    else:
        tabI = tabI.astype(c.npdt)
    a3tF = np.ascontiguousarray(A[3].T)  # [E, V] f32
    if c.z_fp8:
        a3tF = a3tF * 16.0

    j = np.arange(1, J + 1, dtype=np.float32)
    av = 1.0 - j / J
    bv = 2.0 * j / J - 1.0
    sp = np.arange(128) // J
    jj = np.arange(128) % J
    wab = np.zeros((128, 2 * c.SPT), np.float32)
    wabc = np.zeros((128, 3 * c.SPT), np.float32)
    wc = np.zeros((128, c.SPT), np.float32)
    for p in range(128):
        wab[p, 2 * sp[p] + 0] = av[jj[p]]
        wab[p, 2 * sp[p] + 1] = bv[jj[p]]
        wabc[p, 3 * sp[p] + 0] = av[jj[p]]
        wabc[p, 3 * sp[p] + 1] = bv[jj[p]]
        wabc[p, 3 * sp[p] + 2] = 1.0
        wc[p, sp[p]] = 1.0
    wq = np.zeros((128, c.Bc), np.float32)
    for p in range(128):
        wq[p, p // c.QW] = 1.0

    tat = np.ascontiguousarray(TA[0, :S, :].T)   # [E, S]
    tct = np.ascontiguousarray(TC[0, :S, :].T)
    tatn = np.tile(tat, (1, c.Bc))               # [E, NS] batch-major
    tctn = np.tile(tct, (1, c.Bc))
    kp = ((np.arange(E, dtype=np.float32) + 1.0) / E).reshape(E, 1)
    mask2 = np.full((128, c.nblk * c.Bc), NEG, np.float32)
    for k in range(c.nblk):
        for p in range(128):
            s = k * 128 + p
            if s < c.NS:
                mask2[p, k * c.Bc + s // S] = 0.0
    onesc = np.ones((128, 1), np.float32)
    ones1 = np.ones((1, 128), np.float32)

    common = {
        "wab": (wab * wscale).astype(c.gnp),
        "wabc": (wabc * wscale).astype(c.gnp),
        "wc": (wc * wscale).astype(c.gnp),
        "wq": (wq * wscale).astype(c.gnp),
        "tatn": np.ascontiguousarray(tatn),
        "tctn": np.ascontiguousarray(tctn),
        "kp": kp, "mask2": mask2,
        "onesc": onesc.astype(c.npdt), "ones1": ones1,
    }

    nch = len(c.gsizes)
    in_maps = []
    for cc in range(c.ncore):
        xc = x[cc * c.Bc:(cc + 1) * c.Bc].reshape(-1)
        qc = q[cc * c.Bc:(cc + 1) * c.Bc].reshape(-1)
        xq = np.concatenate([xc, qc])
        uniq, rel = np.unique(xq, return_inverse=True)
        assert len(uniq) <= c.ucap, (len(uniq), c.ucap)
        tabc = np.zeros((c.ucap, 4 * E), c.gnp)
        tabc[:len(uniq)] = tabI[uniq]
        rel = rel.astype(np.int16)
        idx = np.zeros((nch, 128, c.gchunk // 16), np.int16)
        off = 0
        for g, gs in enumerate(c.gsizes):
            v = rel[off:off + gs]
            off += gs
            wrapped = v.reshape(-1, 16).T
            idx[g, :, : gs // 16] = np.tile(wrapped, (8, 1))
        a3c = np.ascontiguousarray(
            a3tF[:, cc * c.VSH:(cc + 1) * c.VSH]).astype(c.znp)
        m = dict(common)
        m.update({"tabc": tabc, "idx": idx, "a3t": a3c})
        in_maps.append(m)
    return in_maps


_CACHE = {}


def _get_module(cfg):
    k = cfg.key()
    if k not in _CACHE:
        _CACHE[k] = build_module(cfg)
    return _CACHE[k]


def run(cfg, inputs, trace=False):
    nc = _get_module(cfg)
    in_maps = host_prep(cfg, inputs["x"], inputs["q"], inputs["A"],
                        inputs["TA"], inputs["TC"])
    res = bass_utils.run_bass_kernel_spmd(
        nc, in_maps, core_ids=list(range(cfg.ncore)), trace=trace)
    out = np.concatenate(
        [np.asarray(res.results[cc]["o"]).astype(np.float32)
         for cc in range(cfg.ncore)], axis=1)
    return out, res


def kernel(**inputs) -> np.ndarray:
    cfg = Cfg()
    out, _ = run(cfg, inputs, trace=False)
    return out
